# revision 2
# baseline (speedup 1.0000x reference)
"""Cross-layer transcoder kernel for 8 TRN2 NeuronCores.

Sharding: d_transcoder (F=4096) is split 8 ways (512 features per core).
Each core encodes all tokens against its feature slice, computes partial
cross-layer reconstructions for every target layer, and a ReduceScatter
sums the partials; rank i receives target layer i ([B, D] per core,
L == n_cores == 8).  Decoder bias is added post-RS on each core.

Compute dtype: bf16 operands with fp32 PSUM accumulation (1 cycle/row on
the PE).  Partial outputs and the ReduceScatter run in fp32.
"""

import numpy as np
import ml_dtypes

L, B, D, F = 8, 2048, 768, 4096
NCORES = 8
FL = F // NCORES          # 512 features per core
AF = FL // 128            # 4 f-tiles per core
DT = D // 128             # 6 d-tiles
NW = 4                    # token windows for decode + ReduceScatter chunks
WB = B // NW              # 512 tokens per window
EH = 1024                 # encode token chunk per x DMA

_COMPILED_NC = None


def _build_nc():
    import concourse.mybir as mybir
    import concourse.tile as tile
    from concourse import bacc

    dt = mybir.dt
    nc = bacc.Bacc("TRN2", target_bir_lowering=False, debug=False,
                   num_devices=NCORES)

    xt = nc.dram_tensor("xt", [L, D, B], dt.bfloat16, kind="ExternalInput").ap()
    wenc = nc.dram_tensor("wenc", [L, D, FL], dt.bfloat16, kind="ExternalInput").ap()
    benc = nc.dram_tensor("benc", [128, L * AF], dt.float32, kind="ExternalInput").ap()
    wdec = nc.dram_tensor("wdec", [L, FL, L, D], dt.bfloat16, kind="ExternalInput").ap()
    bdec = nc.dram_tensor("bdec", [128, D], dt.float32, kind="ExternalInput").ap()
    out = nc.dram_tensor("out", [B, D], dt.float32, kind="ExternalOutput").ap()

    RELU = mybir.ActivationFunctionType.Relu

    with tile.TileContext(nc) as tc:
        with (
            tc.tile_pool(name="consts", bufs=1) as consts,
            tc.tile_pool(name="featp", bufs=L * AF) as featp,
            tc.tile_pool(name="dram", bufs=1, space="DRAM") as dram,
        ):
            benc_t = consts.tile([128, L * AF], dt.float32, tag="benc_t")
            nc.sync.dma_start(benc_t[:], benc)
            bdec_t = consts.tile([128, D], dt.float32, tag="bdec_t")
            nc.sync.dma_start(bdec_t[:], bdec)

            feats = [
                [featp.tile([128, B], dt.bfloat16, name=f"feat_{l}_{a}",
                            tag="feat", bufs=L * AF) for a in range(AF)]
                for l in range(L)
            ]

            rs_in = [dram.tile([L, WB, D], dt.float32, name=f"rs_in_{w}",
                               tag=f"rsin{w}") for w in range(NW)]
            rs_out = [dram.tile([WB, D], dt.float32, name=f"rs_out_{w}",
                                tag=f"rsout{w}") for w in range(NW)]

            # ---- Phase E: encode all layers/tokens; feats stay in SBUF ----
            with (
                tc.tile_pool(name="encp", bufs=2) as encp,
                tc.tile_pool(name="pep", bufs=3, space="PSUM") as pep,
            ):
                for l in range(L):
                    wenc_t = encp.tile([128, DT, FL], dt.bfloat16,
                                       tag="wenc_t", bufs=2, name=f"wenc_{l}")
                    nc.sync.dma_start(
                        wenc_t[:], wenc[l].rearrange("(k p) f -> p k f", p=128))
                    for h in range(B // EH):
                        xt_t = encp.tile([128, DT, EH], dt.bfloat16,
                                         tag="xt_t", bufs=2, name=f"xt_{l}_{h}")
                        nc.sync.dma_start(
                            xt_t[:],
                            xt[l].rearrange("(k p) b -> p k b", p=128)
                            [:, :, h * EH:(h + 1) * EH])
                        for a in range(AF):
                            for c in range(EH // 512):
                                ps = pep.tile([128, 512], dt.float32,
                                              tag="pe", bufs=3,
                                              name=f"pe_{l}_{h}_{a}_{c}")
                                for k in range(DT):
                                    nc.tensor.matmul(
                                        ps[:],
                                        wenc_t[:, k, a * 128:(a + 1) * 128],
                                        xt_t[:, k, c * 512:(c + 1) * 512],
                                        start=(k == 0), stop=(k == DT - 1))
                                boff = h * EH + c * 512
                                nc.scalar.activation(
                                    feats[l][a][:, boff:boff + 512], ps[:],
                                    RELU,
                                    bias=benc_t[:, l * AF + a:l * AF + a + 1])

            # ---- Phase D: cross-layer decode + chunked ReduceScatter ----
            with (
                tc.tile_pool(name="decp", bufs=3) as decp,
                tc.tile_pool(name="outp", bufs=6) as outp,
                tc.tile_pool(name="postp", bufs=2) as postp,
                tc.tile_pool(name="pdp", bufs=4, space="PSUM") as pdp,
            ):
                for w in range(NW):
                    for j in range(L):
                        pa = [pdp.tile([128, 512], dt.float32, tag="pa",
                                       bufs=4, name=f"pa_{w}_{j}_{s}")
                              for s in range(4)]
                        pb = [pdp.tile([128, 256], dt.float32, tag="pb",
                                       bufs=4, name=f"pb_{w}_{j}_{s}")
                              for s in range(4)]
                        for l in range(j + 1):
                            wd = decp.tile([128, AF, D], dt.bfloat16,
                                           tag="wd", bufs=3,
                                           name=f"wd_{w}_{j}_{l}")
                            nc.sync.dma_start(
                                wd[:],
                                wdec[l, :, j, :]
                                .rearrange("(a p) d -> p a d", p=128))
                            st = (l == 0)
                            sp = (l == j)
                            for a in range(AF):
                                for s in range(4):
                                    lhsT = feats[l][a][:, w * WB + s * 128:
                                                       w * WB + (s + 1) * 128]
                                    nc.tensor.matmul(
                                        pa[s][:], lhsT, wd[:, a, 0:512],
                                        start=(st and a == 0),
                                        stop=(sp and a == AF - 1))
                                    nc.tensor.matmul(
                                        pb[s][:], lhsT, wd[:, a, 512:768],
                                        start=(st and a == 0),
                                        stop=(sp and a == AF - 1))
                        for s in range(4):
                            ot = outp.tile([128, D], dt.float32, tag="ot",
                                           bufs=6, name=f"ot_{w}_{j}_{s}")
                            nc.vector.tensor_copy(ot[:, 0:512], pa[s][:])
                            nc.scalar.activation(
                                ot[:, 512:768], pb[s][:],
                                mybir.ActivationFunctionType.Copy)
                            nc.sync.dma_start(
                                rs_in[w][j, s * 128:(s + 1) * 128, :], ot[:])
                    nc.gpsimd.collective_compute(
                        "ReduceScatter", mybir.AluOpType.add,
                        replica_groups=[list(range(NCORES))],
                        ins=[rs_in[w].opt()], outs=[rs_out[w].opt()])
                    po = postp.tile([128, WB // 128, D], dt.float32,
                                    tag="po", bufs=2, name=f"po_{w}")
                    nc.sync.dma_start(
                        po[:], rs_out[w].rearrange("(s p) d -> p s d", p=128))
                    nc.vector.tensor_add(
                        po[:], po[:],
                        bdec_t[:, None, :].to_broadcast(po.shape))
                    nc.sync.dma_start(
                        out[w * WB:(w + 1) * WB, :]
                        .rearrange("(s p) d -> p s d", p=128), po[:])

    nc.compile()
    return nc


def _get_nc():
    global _COMPILED_NC
    if _COMPILED_NC is None:
        _COMPILED_NC = _build_nc()
    return _COMPILED_NC


def _make_in_maps(x, W_enc, b_enc, W_dec, b_dec):
    bf16 = ml_dtypes.bfloat16
    x = np.asarray(x, dtype=np.float32)
    W_enc = np.asarray(W_enc, dtype=np.float32)
    b_enc = np.asarray(b_enc, dtype=np.float32)
    W_dec = np.asarray(W_dec, dtype=np.float32)
    b_dec = np.asarray(b_dec, dtype=np.float32)

    xt = np.ascontiguousarray(x.transpose(0, 2, 1)).astype(bf16)  # [L, D, B]
    in_maps = []
    for i in range(NCORES):
        sl = slice(i * FL, (i + 1) * FL)
        wenc_i = np.ascontiguousarray(
            W_enc[:, sl, :].transpose(0, 2, 1)).astype(bf16)      # [L, D, FL]
        benc_i = np.ascontiguousarray(
            b_enc[:, sl].reshape(L, AF, 128).transpose(2, 0, 1)
            .reshape(128, L * AF)).astype(np.float32)             # [128, L*AF]
        wdec_i = np.ascontiguousarray(W_dec[:, sl, :, :]).astype(bf16)
        bdec_i = np.ascontiguousarray(
            np.broadcast_to(b_dec[i][None, :], (128, D))).astype(np.float32)
        in_maps.append({"xt": xt, "wenc": wenc_i, "benc": benc_i,
                        "wdec": wdec_i, "bdec": bdec_i})
    return in_maps


def run(x, W_enc, b_enc, W_dec, b_dec, trace=False):
    """Run the kernel; returns (output [L, B, D] fp32, BassKernelResults)."""
    from concourse import bass_utils

    nc = _get_nc()
    in_maps = _make_in_maps(x, W_enc, b_enc, W_dec, b_dec)
    res = bass_utils.run_bass_kernel_spmd(
        nc, in_maps, core_ids=list(range(NCORES)), trace=trace)
    outs = np.stack([res.results[i]["out"] for i in range(NCORES)], axis=0)
    return np.ascontiguousarray(outs.astype(np.float32)), res


def kernel(x, W_enc, b_enc, W_dec, b_dec):
    out, _ = run(x, W_enc, b_enc, W_dec, b_dec)
    return out
